# revision 1
# baseline (speedup 1.0000x reference)
"""Multi-head attention (B=2, S=4096, D=768, H=12) on 8 trn2 NeuronCores.

Sharding: data-parallel over batch (2) x tensor-parallel over head groups (4):
core c -> batch c//4, heads [3*(c%4), 3*(c%4)+3). Each core projects Q/K/V for
its 3 heads (column-sliced W_q/W_k/W_v), runs flash-style attention in the
transposed (scores^T) domain, applies its row slice of W_o, and a 4-way
ReduceScatter sums the partial outputs, leaving each core with its sequence
quarter of the final output.

All matmul operands are fp16 (1 cyc/row on the PE with fast weight loads;
~2.4e-4 rounding) with fp32 PSUM accumulation. Softmax skips max-subtraction
(scores are provably small: |s|<~2.5) and the denominator is produced by an
extra ones-column in the attn@V stationary.
"""
import contextlib
import ctypes
import sys
import types

import numpy as np

# ---------------------------------------------------------------------------
# NTFF profile hook (image's antenv lacks axon_hooks; install shim so
# run_bass_kernel_spmd(trace=True) can capture exec_time_ns).
# ---------------------------------------------------------------------------
def _install_ntff_hook():
    try:
        from antenv.axon_hooks import get_axon_ntff_profile_hook  # noqa: F401
        return
    except ImportError:
        pass
    import antenv

    mod = types.ModuleType("antenv.axon_hooks")
    _state = {"hook": None}
    mod.set_axon_ntff_profile_hook = lambda h: _state.__setitem__("hook", h)
    mod.get_axon_ntff_profile_hook = lambda: _state["hook"]
    sys.modules["antenv.axon_hooks"] = mod
    antenv.axon_hooks = mod

    try:
        lib = ctypes.CDLL("/opt/axon/libaxon_pjrt.so")
    except OSError:
        return
    if not hasattr(lib, "axon_start_nrt_profile"):
        return
    lib.axon_start_nrt_profile.argtypes = [ctypes.POINTER(ctypes.c_int64), ctypes.c_size_t]
    lib.axon_start_nrt_profile.restype = ctypes.c_int64
    lib.axon_stop_nrt_profile.argtypes = [ctypes.c_char_p]
    lib.axon_stop_nrt_profile.restype = ctypes.c_int64

    @contextlib.contextmanager
    def _hook(output_dir, device_ids):
        import jax

        jax.devices()
        if device_ids:
            ids = (ctypes.c_int64 * len(device_ids))(*device_ids)
            rc = lib.axon_start_nrt_profile(ids, len(device_ids))
        else:
            rc = lib.axon_start_nrt_profile(None, 0)
        if rc != 0:
            raise RuntimeError(f"axon_start_nrt_profile rc={rc}")
        try:
            yield
        finally:
            n = lib.axon_stop_nrt_profile(str(output_dir).encode())
            print(f"ntff profile: {n} file(s) -> {output_dir}", file=sys.stderr)

    mod.set_axon_ntff_profile_hook(_hook)


_install_ntff_hook()

import concourse.bass as bass  # noqa: E402
import concourse.tile as tile  # noqa: E402
from concourse import bacc, bass_utils, mybir  # noqa: E402
from concourse.masks import make_identity  # noqa: E402

f32 = mybir.dt.float32
f16 = mybir.dt.float16
AF = mybir.ActivationFunctionType

B, S, D = 2, 4096, 768
H, DH = 12, 64
NCORES = 8
HPC = 3               # heads per core
E = HPC * DH          # 192: per-core projection width
EP = 256              # padded V projection width (N>=256 keeps fp32r at full rate)
NQC = 4               # q chunks of 1024
QC = S // NQC         # 1024
NST = S // 128        # 32 s-tiles
NCH = S // 512        # 8 projection chunks


def _build_nc():
    nc = bacc.Bacc("TRN2", target_bir_lowering=False, debug=False, num_devices=NCORES)
    xq = nc.dram_tensor("xq", [S, D], f32, kind="ExternalInput").ap()
    xk = nc.dram_tensor("xk", [S, D], f32, kind="ExternalInput").ap()
    xv = nc.dram_tensor("xv", [S, D], f32, kind="ExternalInput").ap()
    wqT = nc.dram_tensor("wqT", [D, E], f32, kind="ExternalInput").ap()
    wkT = nc.dram_tensor("wkT", [D, E], f32, kind="ExternalInput").ap()
    wvT = nc.dram_tensor("wvT", [D, EP], f32, kind="ExternalInput").ap()
    woT = nc.dram_tensor("woT", [E, D], f32, kind="ExternalInput").ap()
    y = nc.dram_tensor("y", [S // 4, D], f32, kind="ExternalOutput").ap()

    with tile.TileContext(nc) as tc:
        _body(tc, xq, xk, xv, wqT, wkT, wvT, woT, y)
    nc.compile()
    return nc


def _body(tc, xq, xk, xv, wqT, wkT, wvT, woT, y):
    nc = tc.nc
    with contextlib.ExitStack() as ctx:
        const = ctx.enter_context(tc.tile_pool(name="const", bufs=1))
        big = ctx.enter_context(tc.tile_pool(name="big", bufs=1))
        xload_p = ctx.enter_context(tc.tile_pool(name="xload", bufs=8))
        strip_p = ctx.enter_context(tc.tile_pool(name="strip", bufs=12))
        expt_p = ctx.enter_context(tc.tile_pool(name="expt", bufs=3))
        small_p = ctx.enter_context(tc.tile_pool(name="small", bufs=2))
        ysb_p = ctx.enter_context(tc.tile_pool(name="ysb", bufs=2))
        ps_s = ctx.enter_context(tc.tile_pool(name="ps_s", bufs=2, space="PSUM"))
        ps_o = ctx.enter_context(tc.tile_pool(name="ps_o", bufs=1, space="PSUM"))
        dram = ctx.enter_context(tc.tile_pool(name="dram", bufs=1, space="DRAM"))

        # ---- constants ----
        ident = const.tile([128, 128], f16)
        make_identity(nc, ident[:])
        ones12 = const.tile([128, 12], f32)
        nc.any.memset(ones12[:], 1.0)
        ones_rf = const.tile([1, 64], f32)
        nc.any.memset(ones_rf[:], 1.0)
        ones_r = const.tile([1, 64], f16)
        nc.vector.tensor_copy(ones_r[:], ones_rf[:])

        # ---- weights -> SBUF f16 via cast-DMA ----
        wq_r = big.tile([128, 6 * E], f16)
        wk_r = big.tile([128, 6 * E], f16)
        wv_r = big.tile([128, 6 * EP], f16)
        wo_r0 = big.tile([128, D], f16)          # woT rows 0-127
        wo_r1 = big.tile([64, D], f16)           # woT rows 128-191
        for w_dram, w_sb, width in ((wqT, wq_r, E), (wkT, wk_r, E), (wvT, wv_r, EP)):
            for j in range(6):
                nc.gpsimd.dma_start(w_sb[:, j * width:(j + 1) * width],
                                    w_dram[j * 128:(j + 1) * 128, :])
        nc.gpsimd.dma_start(wo_r0[:], woT[0:128, :])
        nc.gpsimd.dma_start(wo_r1[:], woT[128:192, :])

        # ---- persistent per-chunk activation tiles (f16) ----
        KT0c = [big.tile([128, 512], f16, tag=f"kt0_{c}", name=f"kt0_{c}") for c in range(NCH)]
        KT1c = [big.tile([64, 512], f16, tag=f"kt1_{c}", name=f"kt1_{c}") for c in range(NCH)]
        QT0c = [big.tile([128, QC], f16, tag=f"qt0_{q}", name=f"qt0_{q}") for q in range(NQC)]
        QT1c = [big.tile([64, QC], f16, tag=f"qt1_{q}", name=f"qt1_{q}") for q in range(NQC)]
        OT0c = [big.tile([128, QC], f16, tag=f"ot0_{q}", name=f"ot0_{q}") for q in range(NQC)]
        OT1c = [big.tile([64, QC], f16, tag=f"ot1_{q}", name=f"ot1_{q}") for q in range(NQC)]
        VONc = [big.tile([128, 4 * HPC * 65], f16, tag=f"von_{c}", name=f"von_{c}") for c in range(NCH)]

        def load_chunk(x_dram, c):
            xt = []
            for st in range(4):
                t = xload_p.tile([128, D], f16, tag="xload")
                nc.gpsimd.dma_start(t[:], x_dram[c * 512 + st * 128:c * 512 + (st + 1) * 128, :])
                xt.append(t)
            return xt

        def transpose_strips(xt):
            strips = []
            for dt in range(6):
                tp = ps_s.tile([128, 512], f16, tag="s")
                for st in range(4):
                    nc.tensor.transpose(tp[:, st * 128:(st + 1) * 128],
                                        xt[st][:, dt * 128:(dt + 1) * 128], ident[:])
                sb = strip_p.tile([128, 512], f16, tag="strip")
                nc.vector.tensor_copy(sb[:], tp[:])
                strips.append(sb)
            return strips

        def proj_T(strips, w_sb, dst0, dst1, col0, ncols):
            # dst0[:, col0:col0+ncols] = (W rows 0-127)^T contraction, dst1 rows 128-191
            for ep, (lo, sz, dst) in enumerate(((0, 128, dst0), (128, 64, dst1))):
                pp = ps_s.tile([128, 512], f32, tag="s")
                for dt in range(6):
                    nc.tensor.matmul(pp[0:sz, 0:ncols],
                                     w_sb[:, dt * E + lo:dt * E + lo + sz],
                                     strips[dt][:, 0:ncols],
                                     start=(dt == 0), stop=(dt == 5))
                nc.vector.tensor_copy(dst[0:sz, col0:col0 + ncols], pp[0:sz, 0:ncols])

        # ---- phase A: K, V for all chunks; then Q ----
        for c in range(NCH):
            kt = load_chunk(xk, c)
            vt = load_chunk(xv, c)
            kstrips = transpose_strips(kt)
            proj_T(kstrips, wk_r, KT0c[c], KT1c[c], 0, 512)
            vstrips = transpose_strips(vt)
            von = VONc[c]
            v3 = von[:].rearrange("p (h c) -> p h c", c=65)
            for st in range(4):
                pp = ps_s.tile([128, EP], f32, tag="s")
                for dt in range(6):
                    nc.tensor.matmul(pp[:], vstrips[dt][:, st * 128:(st + 1) * 128],
                                     wv_r[:, dt * EP:(dt + 1) * EP],
                                     start=(dt == 0), stop=(dt == 5))
                nc.vector.tensor_copy(v3[:, st * HPC:(st + 1) * HPC, 0:64],
                                      pp[:, 0:E].rearrange("p (h c) -> p h c", c=64))
            nc.vector.tensor_copy(v3[:, :, 64:65],
                                  ones12[:].rearrange("p (h c) -> p h c", c=1))

        for qc in range(NQC):
            for half in range(2):
                c = 2 * qc + half
                qt = load_chunk(xq, c)
                qstrips = transpose_strips(qt)
                proj_T(qstrips, wq_r, QT0c[qc], QT1c[qc], half * 512, 512)

        # ---- phase B: attention + Wo + chunked ReduceScatter ----
        for qc in range(NQC):
            # ---- heads 0,1: q-chunks of 512; every consecutive PE op alternates
            # row groups (base 0 / base 64) so matmuls pack and pipeline.
            for half8 in range(2):
                q0 = qc * QC + half8 * 512
                poA_lo = ps_o.tile([65, 512], f32, tag="po_al", bufs=1, name=f"poal_{qc}_{half8}")
                poA_hi = ps_o.tile([65, 512], f32, tag="po_ah", bufs=1, name=f"poah_{qc}_{half8}")
                poB_lo = ps_o.tile([65, 512], f32, tag="po_bl", bufs=1, name=f"pobl_{qc}_{half8}")
                poB_hi = ps_o.tile([65, 512], f32, tag="po_bh", bufs=1, name=f"pobh_{qc}_{half8}")
                qsl = slice(half8 * 512, (half8 + 1) * 512)
                for t in range(NST):
                    kc, stl = t // 4, t % 4
                    psAB = ps_s.tile([128, 1024], f32, tag="s", name=f"psAB_{qc}_{half8}_{t}")
                    nc.tensor.matmul(psAB[:, 0:512], KT0c[kc][0:64, stl * 128:(stl + 1) * 128],
                                     QT0c[qc][0:64, qsl], start=True, stop=True)
                    nc.tensor.matmul(psAB[:, 512:1024], KT0c[kc][64:128, stl * 128:(stl + 1) * 128],
                                     QT0c[qc][64:128, qsl], start=True, stop=True)
                    et = expt_p.tile([128, 1024], f16, tag="expt", name=f"et_{qc}_{half8}_{t}")
                    nc.scalar.activation(et[:], psAB[:], AF.Exp, scale=0.125)
                    vA = VONc[kc][:, (stl * HPC + 0) * 65:(stl * HPC + 1) * 65]
                    vB = VONc[kc][:, (stl * HPC + 1) * 65:(stl * HPC + 2) * 65]
                    st_fl = dict(start=(t == 0), stop=(t == NST - 1))
                    nc.tensor.matmul(poA_lo[:], vA[0:64, :], et[0:64, 0:512], **st_fl)
                    nc.tensor.matmul(poA_hi[:], vA[64:128, :], et[64:128, 0:512], **st_fl)
                    nc.tensor.matmul(poB_lo[:], vB[0:64, :], et[0:64, 512:1024], **st_fl)
                    nc.tensor.matmul(poB_hi[:], vB[64:128, :], et[64:128, 512:1024], **st_fl)

                def normalize(po_lo, po_hi, dst, via_dma):
                    oev_lo = small_p.tile([65, 512], f32, tag="oevl")
                    nc.vector.tensor_copy(oev_lo[:], po_lo[:])
                    oev = small_p.tile([65, 512], f32, tag="oev")
                    nc.vector.tensor_add(oev[:], po_hi[:], oev_lo[:])
                    rc = small_p.tile([1, 512], f16, tag="recip")
                    with nc.allow_low_precision(reason="softmax denominator"):
                        nc.vector.reciprocal(rc[:], oev[64:65, :])
                    pb = ps_s.tile([128, 512], f32, tag="s", name="pb_norm")
                    nc.tensor.matmul(pb[0:64, :], ones_r[:], rc[:], start=True, stop=True)
                    if via_dma:
                        nrm = small_p.tile([64, 512], f16, tag="nrm")
                        nc.vector.tensor_mul(nrm[:], oev[0:64, :], pb[0:64, :])
                        nc.sync.dma_start(dst, nrm[:])
                    else:
                        nc.vector.tensor_mul(dst, oev[0:64, :], pb[0:64, :])

                normalize(poA_lo, poA_hi, OT0c[qc][0:64, qsl], False)
                normalize(poB_lo, poB_hi, OT0c[qc][64:128, qsl], True)

                # head 2 (single stream; scores base0 / attnV halves alternate)
                poC_lo = ps_o.tile([65, 512], f32, tag="po_al", bufs=1, name=f"pocl_{qc}_{half8}")
                poC_hi = ps_o.tile([65, 512], f32, tag="po_ah", bufs=1, name=f"poch_{qc}_{half8}")
                for t in range(NST):
                    kc, stl = t // 4, t % 4
                    psC = ps_s.tile([128, 512], f32, tag="s", name=f"psC_{qc}_{half8}_{t}")
                    nc.tensor.matmul(psC[:], KT1c[kc][0:64, stl * 128:(stl + 1) * 128],
                                     QT1c[qc][0:64, qsl], start=True, stop=True)
                    etC = expt_p.tile([128, 512], f16, tag="expt", name=f"etC_{qc}_{half8}_{t}")
                    nc.scalar.activation(etC[:], psC[:], AF.Exp, scale=0.125)
                    vC = VONc[kc][:, (stl * HPC + 2) * 65:(stl * HPC + 3) * 65]
                    st_fl = dict(start=(t == 0), stop=(t == NST - 1))
                    nc.tensor.matmul(poC_hi[:], vC[64:128, :], etC[64:128, :], **st_fl)
                    nc.tensor.matmul(poC_lo[:], vC[0:64, :], etC[0:64, :], **st_fl)
                normalize(poC_lo, poC_hi, OT1c[qc][0:64, qsl], False)

            # ---- Wo for this qc + chunked ReduceScatter
            nblk = 2 if qc < 3 else 4
            bsz = 8 // nblk           # s-tiles per block
            for blk in range(nblk):
                rs_in = dram.tile([bsz * 128, D], f32, tag=f"rsin_{qc}_{blk}", name=f"rsin_{qc}_{blk}")
                rs_out = dram.tile([bsz * 32, D], f32, tag=f"rsout_{qc}_{blk}", name=f"rsout_{qc}_{blk}")
                for sl in range(bsz):
                    stl = blk * bsz + sl
                    py0 = ps_s.tile([128, 512], f32, tag="s", name=f"py0_{qc}_{blk}_{sl}")
                    py1 = ps_s.tile([128, 256], f32, tag="s", name=f"py1_{qc}_{blk}_{sl}")
                    for py, e0, esz in ((py0, 0, 512), (py1, 512, 256)):
                        nc.tensor.matmul(py[:, 0:esz],
                                         OT0c[qc][:, stl * 128:(stl + 1) * 128],
                                         wo_r0[:, e0:e0 + esz], start=True, stop=False)
                        nc.tensor.matmul(py[:, 0:esz],
                                         OT1c[qc][0:64, stl * 128:(stl + 1) * 128],
                                         wo_r1[:, e0:e0 + esz], start=False, stop=True)
                    ys = ysb_p.tile([128, D], f32, tag="ysb")
                    nc.vector.tensor_copy(ys[:, 0:512], py0[:])
                    nc.vector.tensor_copy(ys[:, 512:768], py1[:])
                    nc.sync.dma_start(rs_in[sl * 128:(sl + 1) * 128, :], ys[:])
                nc.gpsimd.collective_compute(
                    "ReduceScatter",
                    mybir.AluOpType.add,
                    replica_groups=[[0, 1, 2, 3], [4, 5, 6, 7]],
                    ins=[rs_in.opt()],
                    outs=[rs_out.opt()],
                )
                yoff = qc * 256 + blk * bsz * 32
                nc.sync.dma_start(y[yoff:yoff + bsz * 32, :], rs_out[:])


_NC_CACHE = None


def _get_nc():
    global _NC_CACHE
    if _NC_CACHE is None:
        _NC_CACHE = _build_nc()
    return _NC_CACHE


def _make_in_maps(query, key, value, W_q, W_k, W_v, W_o):
    query = np.asarray(query, dtype=np.float32)
    key = np.asarray(key, dtype=np.float32)
    value = np.asarray(value, dtype=np.float32)
    wq_t = np.ascontiguousarray(np.asarray(W_q, np.float32).T)  # [d_in, e_out]
    wk_t = np.ascontiguousarray(np.asarray(W_k, np.float32).T)
    wv_t = np.ascontiguousarray(np.asarray(W_v, np.float32).T)
    wo_t = np.ascontiguousarray(np.asarray(W_o, np.float32).T)  # [d_in(heads), e_out]
    in_maps = []
    for c in range(NCORES):
        b, g = c // 4, c % 4
        sl = slice(g * E, (g + 1) * E)
        wv_pad = np.zeros((D, EP), np.float32)
        wv_pad[:, 0:E] = wv_t[:, sl]
        in_maps.append({
            "xq": np.ascontiguousarray(query[b]),
            "xk": np.ascontiguousarray(key[b]),
            "xv": np.ascontiguousarray(value[b]),
            "wqT": np.ascontiguousarray(wq_t[:, sl]),
            "wkT": np.ascontiguousarray(wk_t[:, sl]),
            "wvT": wv_pad,
            "woT": np.ascontiguousarray(wo_t[sl, :]),
        })
    return in_maps


def run(in_maps, trace=False):
    nc = _get_nc()
    return bass_utils.run_bass_kernel_spmd(
        nc, in_maps, core_ids=list(range(NCORES)), trace=trace)


def assemble(results):
    # qc 0-2: two 512-row RS blocks (128 rows/core); qc 3: four 256-row
    # blocks (64 rows/core) for a smaller collective tail.
    out = np.empty((B, S, D), np.float32)
    for c in range(NCORES):
        b, g = c // 4, c % 4
        yc = results[c]["y"]
        yo = 0
        for qc in range(NQC):
            nblk = 2 if qc < 3 else 4
            gsz = 512 // nblk * 2 // 4 * 4 // 4  # rows per core per block
            gsz = (1024 // nblk) // 4
            for blk in range(nblk):
                g0 = qc * 1024 + blk * (1024 // nblk) + g * gsz
                out[b, g0:g0 + gsz] = yc[yo:yo + gsz]
                yo += gsz
    return out


def kernel(**inputs):
    in_maps = _make_in_maps(**inputs)
    res = run(in_maps)
    return assemble(res.results)



# revision 3
# speedup vs baseline: 1.4577x; 1.4577x over previous
"""Multi-head attention (B=2, S=4096, D=768, H=12) on 8 trn2 NeuronCores.

Sharding: data-parallel over batch (2) x tensor-parallel over head groups (4):
core c -> batch c//4, heads [3*(c%4), 3*(c%4)+3). Each core projects Q/K/V for
its 3 heads (column-sliced W_q/W_k/W_v), runs flash-style attention in the
transposed (scores^T) domain, applies its row slice of W_o, and a 4-way f16
ReduceScatter sums the partial outputs, leaving each core with its sequence
quarter of the final output.

All matmul operands are fp16 (1 cyc/row on the PE with fast weight loads;
~2.4e-4 rounding) with fp32 PSUM accumulation. Softmax skips max-subtraction
(scores are provably small: |s|<~2.5) and the denominator is produced by an
extra ones-column in the attn@V stationary. attn@V contracts the full 128
kpos partitions per matmul (K=128), so the attention inner loops are bound by
the scalar-engine EXP, not the PE. Normalization and W_o run one pipeline
stage behind attention so the in-order tensor queue never stalls on
reciprocals or the collective.
"""
import contextlib
import ctypes
import sys
import types

import numpy as np

# ---------------------------------------------------------------------------
# NTFF profile hook (image's antenv lacks axon_hooks; install shim so
# run_bass_kernel_spmd(trace=True) can capture exec_time_ns).
# ---------------------------------------------------------------------------
def _install_ntff_hook():
    try:
        from antenv.axon_hooks import get_axon_ntff_profile_hook  # noqa: F401
        return
    except ImportError:
        pass
    import antenv

    mod = types.ModuleType("antenv.axon_hooks")
    _state = {"hook": None}
    mod.set_axon_ntff_profile_hook = lambda h: _state.__setitem__("hook", h)
    mod.get_axon_ntff_profile_hook = lambda: _state["hook"]
    sys.modules["antenv.axon_hooks"] = mod
    antenv.axon_hooks = mod

    try:
        lib = ctypes.CDLL("/opt/axon/libaxon_pjrt.so")
    except OSError:
        return
    if not hasattr(lib, "axon_start_nrt_profile"):
        return
    lib.axon_start_nrt_profile.argtypes = [ctypes.POINTER(ctypes.c_int64), ctypes.c_size_t]
    lib.axon_start_nrt_profile.restype = ctypes.c_int64
    lib.axon_stop_nrt_profile.argtypes = [ctypes.c_char_p]
    lib.axon_stop_nrt_profile.restype = ctypes.c_int64

    @contextlib.contextmanager
    def _hook(output_dir, device_ids):
        import jax

        jax.devices()
        if device_ids:
            ids = (ctypes.c_int64 * len(device_ids))(*device_ids)
            rc = lib.axon_start_nrt_profile(ids, len(device_ids))
        else:
            rc = lib.axon_start_nrt_profile(None, 0)
        if rc != 0:
            raise RuntimeError(f"axon_start_nrt_profile rc={rc}")
        try:
            yield
        finally:
            n = lib.axon_stop_nrt_profile(str(output_dir).encode())
            print(f"ntff profile: {n} file(s) -> {output_dir}", file=sys.stderr)

    mod.set_axon_ntff_profile_hook(_hook)


_install_ntff_hook()

import concourse.bass as bass  # noqa: E402
import concourse.tile as tile  # noqa: E402
from concourse import bacc, bass_utils, mybir  # noqa: E402
from concourse.masks import make_identity  # noqa: E402

f32 = mybir.dt.float32
f16 = mybir.dt.float16
AF = mybir.ActivationFunctionType

B, S, D = 2, 4096, 768
H, DH = 12, 64
NCORES = 8
HPC = 3               # heads per core
E = HPC * DH          # 192: per-core projection width
EP = 256              # padded V projection width (N>=256 keeps fp32r at full rate)
NQC = 4               # q chunks of 1024
QC = S // NQC         # 1024
NST = S // 128        # 32 s-tiles
NCH = S // 512        # 8 projection chunks
RG = [[0, 1, 2, 3], [4, 5, 6, 7]]


def _build_nc():
    nc = bacc.Bacc("TRN2", target_bir_lowering=False, debug=False, num_devices=NCORES)
    xq = nc.dram_tensor("xq", [S, D], f32, kind="ExternalInput").ap()
    xk = nc.dram_tensor("xk", [S, D], f32, kind="ExternalInput").ap()
    xv = nc.dram_tensor("xv", [S, D], f32, kind="ExternalInput").ap()
    wqT = nc.dram_tensor("wqT", [D, E], f32, kind="ExternalInput").ap()
    wkT = nc.dram_tensor("wkT", [D, E], f32, kind="ExternalInput").ap()
    wvT = nc.dram_tensor("wvT", [D, EP], f32, kind="ExternalInput").ap()
    woT = nc.dram_tensor("woT", [E, D], f32, kind="ExternalInput").ap()
    y = nc.dram_tensor("y", [S // 4, D], f16, kind="ExternalOutput").ap()

    with tile.TileContext(nc) as tc:
        _body(tc, xq, xk, xv, wqT, wkT, wvT, woT, y)
    nc.compile()
    return nc


def _body(tc, xq, xk, xv, wqT, wkT, wvT, woT, y):
    nc = tc.nc
    with contextlib.ExitStack() as ctx:
        const = ctx.enter_context(tc.tile_pool(name="const", bufs=1))
        big = ctx.enter_context(tc.tile_pool(name="big", bufs=1))
        xload_p = ctx.enter_context(tc.tile_pool(name="xload", bufs=8))
        strip_p = ctx.enter_context(tc.tile_pool(name="strip", bufs=12))
        expt_p = ctx.enter_context(tc.tile_pool(name="expt", bufs=3))
        oev_p = ctx.enter_context(tc.tile_pool(name="oev", bufs=12))
        rc_p = ctx.enter_context(tc.tile_pool(name="rc", bufs=16))
        ysb_p = ctx.enter_context(tc.tile_pool(name="ysb", bufs=2))
        ps_s = ctx.enter_context(tc.tile_pool(name="ps_s", bufs=2, space="PSUM"))
        ps_o = ctx.enter_context(tc.tile_pool(name="ps_o", bufs=1, space="PSUM"))
        dram = ctx.enter_context(tc.tile_pool(name="dram", bufs=1, space="DRAM"))

        # ---- constants ----
        ident = const.tile([128, 128], f16)
        make_identity(nc, ident[:])
        ones12 = const.tile([128, 12], f32)
        nc.any.memset(ones12[:], 1.0)
        ones_rf = const.tile([1, 64], f32)
        nc.any.memset(ones_rf[:], 1.0)
        ones_r = const.tile([1, 64], f16)
        nc.vector.tensor_copy(ones_r[:], ones_rf[:])

        # ---- weights -> SBUF f16 via cast-DMA ----
        wq_r = big.tile([128, 6 * E], f16)
        wk_r = big.tile([128, 6 * E], f16)
        wv_r = big.tile([128, 6 * EP], f16)
        wo_r0 = big.tile([128, D], f16)          # woT rows 0-127
        wo_r1 = big.tile([64, D], f16)           # woT rows 128-191
        for w_dram, w_sb, width in ((wqT, wq_r, E), (wkT, wk_r, E), (wvT, wv_r, EP)):
            for j in range(6):
                nc.gpsimd.dma_start(w_sb[:, j * width:(j + 1) * width],
                                    w_dram[j * 128:(j + 1) * 128, :])
        nc.gpsimd.dma_start(wo_r0[:], woT[0:128, :])
        nc.gpsimd.dma_start(wo_r1[:], woT[128:192, :])

        # ---- persistent per-chunk activation tiles (f16) ----
        KT0c = [big.tile([128, 512], f16, tag=f"kt0_{c}", name=f"kt0_{c}") for c in range(NCH)]
        KT1c = [big.tile([64, 512], f16, tag=f"kt1_{c}", name=f"kt1_{c}") for c in range(NCH)]
        QT0c = [big.tile([128, QC], f16, tag=f"qt0_{q}", name=f"qt0_{q}") for q in range(NQC)]
        QT1c = [big.tile([64, QC], f16, tag=f"qt1_{q}", name=f"qt1_{q}") for q in range(NQC)]
        OT0c = [big.tile([128, QC], f16, tag=f"ot0_{q}", name=f"ot0_{q}") for q in range(NQC)]
        OT1c = [big.tile([64, QC], f16, tag=f"ot1_{q}", name=f"ot1_{q}") for q in range(NQC)]
        VONc = [big.tile([128, 4 * HPC * 65], f16, tag=f"von_{c}", name=f"von_{c}") for c in range(NCH)]

        def load_chunk(x_dram, c):
            xt = []
            for st in range(4):
                t = xload_p.tile([128, D], f16, tag="xload")
                nc.gpsimd.dma_start(t[:], x_dram[c * 512 + st * 128:c * 512 + (st + 1) * 128, :])
                xt.append(t)
            return xt

        def transpose_strips(xt):
            strips = []
            for dt in range(6):
                tp = ps_s.tile([128, 512], f16, tag="s")
                for st in range(4):
                    nc.tensor.transpose(tp[:, st * 128:(st + 1) * 128],
                                        xt[st][:, dt * 128:(dt + 1) * 128], ident[:])
                sb = strip_p.tile([128, 512], f16, tag="strip")
                nc.vector.tensor_copy(sb[:], tp[:])
                strips.append(sb)
            return strips

        def proj_T(strips, w_sb, dst0, dst1, col0, ncols):
            # dst0[:, col0:col0+ncols] = (W rows 0-127)^T contraction, dst1 rows 128-191
            for ep, (lo, sz, dst) in enumerate(((0, 128, dst0), (128, 64, dst1))):
                pp = ps_s.tile([128, 512], f32, tag="s")
                for dt in range(6):
                    nc.tensor.matmul(pp[0:sz, 0:ncols],
                                     w_sb[:, dt * E + lo:dt * E + lo + sz],
                                     strips[dt][:, 0:ncols],
                                     start=(dt == 0), stop=(dt == 5))
                nc.vector.tensor_copy(dst[0:sz, col0:col0 + ncols], pp[0:sz, 0:ncols])

        # ---- phase A: K, V for all chunks; then Q ----
        for c in range(NCH):
            kt = load_chunk(xk, c)
            vt = load_chunk(xv, c)
            kstrips = transpose_strips(kt)
            proj_T(kstrips, wk_r, KT0c[c], KT1c[c], 0, 512)
            vstrips = transpose_strips(vt)
            von = VONc[c]
            v3 = von[:].rearrange("p (h c) -> p h c", c=65)
            for st in range(4):
                pp = ps_s.tile([128, EP], f32, tag="s")
                for dt in range(6):
                    nc.tensor.matmul(pp[:], vstrips[dt][:, st * 128:(st + 1) * 128],
                                     wv_r[:, dt * EP:(dt + 1) * EP],
                                     start=(dt == 0), stop=(dt == 5))
                nc.vector.tensor_copy(v3[:, st * HPC:(st + 1) * HPC, 0:64],
                                      pp[:, 0:E].rearrange("p (h c) -> p h c", c=64))
            nc.vector.tensor_copy(v3[:, :, 64:65],
                                  ones12[:].rearrange("p (h c) -> p h c", c=1))

        for qc in range(NQC):
            for half in range(2):
                c = 2 * qc + half
                qt = load_chunk(xq, c)
                qstrips = transpose_strips(qt)
                proj_T(qstrips, wq_r, QT0c[qc], QT1c[qc], half * 512, 512)

        # ---- phase B: attention (EXP-bound loops), pipelined normalize+Wo+RS ----
        def attn_AB(qc, half8):
            qsl = slice(half8 * 512, (half8 + 1) * 512)
            poA = ps_o.tile([65, 512], f32, tag="poA", bufs=1, name=f"poA_{qc}_{half8}")
            poB = ps_o.tile([65, 512], f32, tag="poB", bufs=1, name=f"poB_{qc}_{half8}")
            for t in range(NST):
                kc, stl = t // 4, t % 4
                psAB = ps_s.tile([128, 1024], f32, tag="s", name=f"psAB_{qc}_{half8}_{t}")
                nc.tensor.matmul(psAB[:, 0:512], KT0c[kc][0:64, stl * 128:(stl + 1) * 128],
                                 QT0c[qc][0:64, qsl], start=True, stop=True)
                nc.tensor.matmul(psAB[:, 512:1024], KT0c[kc][64:128, stl * 128:(stl + 1) * 128],
                                 QT0c[qc][64:128, qsl], start=True, stop=True)
                et = expt_p.tile([128, 1024], f16, tag="expt", name=f"et_{qc}_{half8}_{t}")
                nc.scalar.activation(et[:], psAB[:], AF.Exp, scale=0.125)
                vA = VONc[kc][:, (stl * HPC + 0) * 65:(stl * HPC + 1) * 65]
                vB = VONc[kc][:, (stl * HPC + 1) * 65:(stl * HPC + 2) * 65]
                st_fl = dict(start=(t == 0), stop=(t == NST - 1))
                nc.tensor.matmul(poA[:], vA, et[:, 0:512], **st_fl)
                nc.tensor.matmul(poB[:], vB, et[:, 512:1024], **st_fl)
            return poA, poB

        def attn_C(qc, half8):
            qsl = slice(half8 * 512, (half8 + 1) * 512)
            poC = ps_o.tile([65, 512], f32, tag="poC", bufs=1, name=f"poC_{qc}_{half8}")
            for ti in range(NST // 2):
                pair = (2 * ti, 2 * ti + 1)
                psC = ps_s.tile([128, 1024], f32, tag="s", name=f"psC_{qc}_{half8}_{ti}")
                for j, t in enumerate(pair):
                    kc, stl = t // 4, t % 4
                    nc.tensor.matmul(psC[:, j * 512:(j + 1) * 512],
                                     KT1c[kc][0:64, stl * 128:(stl + 1) * 128],
                                     QT1c[qc][0:64, qsl], start=True, stop=True)
                etC = expt_p.tile([128, 1024], f16, tag="expt", name=f"etC_{qc}_{half8}_{ti}")
                nc.scalar.activation(etC[:], psC[:], AF.Exp, scale=0.125)
                for j, t in enumerate(pair):
                    kc, stl = t // 4, t % 4
                    vC = VONc[kc][:, (stl * HPC + 2) * 65:(stl * HPC + 3) * 65]
                    nc.tensor.matmul(poC[:], vC, etC[:, j * 512:(j + 1) * 512],
                                     start=(t == 0), stop=(t == NST - 1))
            return poC

        def stage_norm(po, qc, half8, h):
            # Drain PSUM immediately: numerator rows -> f16 SBUF, denominator
            # row -> f32 SBUF, then reciprocal (long latency, but consumed a
            # full pipeline stage later so it never stalls the tensor queue).
            oev = oev_p.tile([64, 512], f16, tag="oev", name=f"oev_{qc}_{half8}_{h}")
            nc.vector.tensor_copy(oev[:], po[0:64, :])
            dnm = rc_p.tile([1, 512], f32, tag="dnm", name=f"dnm_{qc}_{half8}_{h}")
            nc.vector.tensor_copy(dnm[:], po[64:65, :])
            rch = rc_p.tile([1, 512], f16, tag="rch", name=f"rch_{qc}_{half8}_{h}")
            with nc.allow_low_precision(reason="softmax denominator"):
                nc.vector.reciprocal(rch[:], dnm[:])
            return oev, rch

        def norm_finish(items, qc, half8):
            # One stage later: broadcast 1/denom across 64 partitions on the
            # PE (reciprocals are long done -> no tensor-queue stall), scale.
            qsl = slice(half8 * 512, (half8 + 1) * 512)
            dsts = (OT0c[qc][0:64, qsl], OT0c[qc][64:128, qsl], OT1c[qc][0:64, qsl])
            for h, (oev, rch) in enumerate(items):
                pb = ps_s.tile([64, 512], f32, tag="s", name=f"pb_{qc}_{half8}_{h}")
                nc.tensor.matmul(pb[:], ones_r[:], rch[:], start=True, stop=True)
                nc.vector.tensor_mul(dsts[h], oev[:], pb[:])

        def wo_rs(qc):
            rs_in = dram.tile([QC, D], f16, tag=f"rsin_{qc}", name=f"rsin_{qc}")
            rs_out = dram.tile([QC // 4, D], f16, tag=f"rsout_{qc}", name=f"rsout_{qc}")
            for stl in range(8):
                py0 = ps_s.tile([128, 512], f32, tag="s", name=f"py0_{qc}_{stl}")
                py1 = ps_s.tile([128, 256], f32, tag="s", name=f"py1_{qc}_{stl}")
                for py, e0, esz in ((py0, 0, 512), (py1, 512, 256)):
                    nc.tensor.matmul(py[:, 0:esz],
                                     OT0c[qc][:, stl * 128:(stl + 1) * 128],
                                     wo_r0[:, e0:e0 + esz], start=True, stop=False)
                    nc.tensor.matmul(py[:, 0:esz],
                                     OT1c[qc][0:64, stl * 128:(stl + 1) * 128],
                                     wo_r1[:, e0:e0 + esz], start=False, stop=True)
                ys = ysb_p.tile([128, D], f16, tag="ysb")
                nc.vector.tensor_copy(ys[:, 0:512], py0[:])
                nc.vector.tensor_copy(ys[:, 512:768], py1[:])
                nc.sync.dma_start(rs_in[stl * 128:(stl + 1) * 128, :], ys[:])
            nc.gpsimd.collective_compute(
                "ReduceScatter",
                mybir.AluOpType.add,
                replica_groups=RG,
                ins=[rs_in.opt()],
                outs=[rs_out.opt()],
            )
            nc.sync.dma_start(y[qc * 256:(qc + 1) * 256, :], rs_out[:])

        sections = [(qc, h8) for qc in range(NQC) for h8 in range(2)]
        pending = {}
        for i, sec in enumerate(sections):
            qc, h8 = sec
            poA, poB = attn_AB(qc, h8)
            items = [stage_norm(poA, qc, h8, 0), stage_norm(poB, qc, h8, 1)]
            poC = attn_C(qc, h8)
            items.append(stage_norm(poC, qc, h8, 2))
            pending[sec] = items
            if i >= 1:
                pqc, ph8 = sections[i - 1]
                norm_finish(pending.pop((pqc, ph8)), pqc, ph8)
                if ph8 == 1:
                    wo_rs(pqc)
        lqc, lh8 = sections[-1]
        norm_finish(pending.pop((lqc, lh8)), lqc, lh8)
        wo_rs(lqc)


_NC_CACHE = None


def _get_nc():
    global _NC_CACHE
    if _NC_CACHE is None:
        _NC_CACHE = _build_nc()
    return _NC_CACHE


def _make_in_maps(query, key, value, W_q, W_k, W_v, W_o):
    query = np.asarray(query, dtype=np.float32)
    key = np.asarray(key, dtype=np.float32)
    value = np.asarray(value, dtype=np.float32)
    wq_t = np.ascontiguousarray(np.asarray(W_q, np.float32).T)  # [d_in, e_out]
    wk_t = np.ascontiguousarray(np.asarray(W_k, np.float32).T)
    wv_t = np.ascontiguousarray(np.asarray(W_v, np.float32).T)
    wo_t = np.ascontiguousarray(np.asarray(W_o, np.float32).T)  # [d_in(heads), e_out]
    in_maps = []
    for c in range(NCORES):
        b, g = c // 4, c % 4
        sl = slice(g * E, (g + 1) * E)
        wv_pad = np.zeros((D, EP), np.float32)
        wv_pad[:, 0:E] = wv_t[:, sl]
        in_maps.append({
            "xq": np.ascontiguousarray(query[b]),
            "xk": np.ascontiguousarray(key[b]),
            "xv": np.ascontiguousarray(value[b]),
            "wqT": np.ascontiguousarray(wq_t[:, sl]),
            "wkT": np.ascontiguousarray(wk_t[:, sl]),
            "wvT": wv_pad,
            "woT": np.ascontiguousarray(wo_t[sl, :]),
        })
    return in_maps


def run(in_maps, trace=False):
    nc = _get_nc()
    return bass_utils.run_bass_kernel_spmd(
        nc, in_maps, core_ids=list(range(NCORES)), trace=trace)


def assemble(results):
    # Per qc, each core holds its group's 256-row quarter of that 1024-row
    # sequence block (f16); cast up on assembly.
    out = np.empty((B, S, D), np.float32)
    for c in range(NCORES):
        b, g = c // 4, c % 4
        yc = results[c]["y"]
        for qc in range(NQC):
            g0 = qc * QC + g * 256
            out[b, g0:g0 + 256] = yc[qc * 256:(qc + 1) * 256].astype(np.float32)
    return out


def kernel(**inputs):
    in_maps = _make_in_maps(**inputs)
    res = run(in_maps)
    return assemble(res.results)
